# revision 18
# baseline (speedup 1.0000x reference)
"""Trainium2 Bass kernel for a cross-attention transformer block.

Contract: kernel(**inputs) takes the FULL inputs (B=8 batch), shards
batch-wise across 8 NeuronCores (one batch element per core, SPMD, no
collectives), runs a Bass/Tile kernel, and returns the FULL output.

v2 design (vs the 177µs fp32r baseline):
  - all GEMM operands bf16 (1 cyc/row on PE at any width, half the DMA
    bytes, ~4.5e-3 end-to-end rel err vs the 2e-2 budget)
  - exact valid lengths: keys kv = max(vl1), query width W = max(vl2)+1
    (no padding to 128); the extra query column is an all-zero query with
    wvec=1, which makes attention produce the shared "masked row" value
    (mean over all 1024 keys of v@Wv) in column W-1
  - phases C/D (Wo, FFN, LayerNorm) run on W tokens instead of 1024;
    output rows W..1024 are a broadcast-DMA of row W-1
  - softmax exp: one ACT instruction per (head, key-tile) over the full
    [rows, W] contiguous PSUM region (bias = per-partition colneg)
  - masked-key aux row (sum of invalid v rows) rides along as an extra
    column of vT through the V projection; extracted from PSUM by DMA
  - per-head pipeline: scores (PE) -> exp (ACT) -> attn@V+denominator
    (PE, M=65 with a ones row) -> rank-1 wvec correction (PE) ->
    reciprocal + broadcast-by-matmul + divide (DVE/PE)
"""

import sys

for _p in ("/opt/trn_rl_repo",):
    if _p not in sys.path:
        sys.path.insert(0, _p)

from contextlib import ExitStack

import numpy as np
import ml_dtypes

import concourse.bacc as bacc
import concourse.tile as tile
from concourse import mybir

F32 = mybir.dt.float32
BF = mybir.dt.bfloat16
U16 = mybir.dt.uint16
AF = mybir.ActivationFunctionType
OP = mybir.AluOpType

D = 768
H = 12
HD = 64
DT = 6          # feature tiles of 128
L = 1024
NEG = -1000000.0
EPS = 1e-5
ONE_BF = 0x3F80  # 1.0 in bfloat16 bits


def _chunks(w):
    """Split width into PSUM-bank-sized matmul chunks (<=512 each)."""
    out, off = [], 0
    while w > 0:
        c = min(512, w)
        out.append((off, c))
        off += c
        w -= c
    return out


def build_program(kv, qv, n_cores, has_d1b=False, has_d2b=False,
                  has_g=False, has_b=False):
    W = min(qv + 1, L)          # query width incl tail column
    qch = _chunks(W)
    kch = _chunks(kv)
    kt_n = (kv + 127) // 128
    vt_n = (kv + 1 + 127) // 128  # vT has kv+1 cols (aux masked-sum)
    aux_t, aux_r = kv // 128, kv % 128
    qt_n = (W + 127) // 128       # token tiles for natural-layout phases

    def krows(i):
        return min(128, kv - 128 * i)

    nc = bacc.Bacc("TRN2", target_bir_lowering=False, debug=False,
                   num_devices=n_cores)

    def din(name, shape, dt=BF):
        return nc.dram_tensor(name, shape, dt, kind="ExternalInput").ap()

    qT = din("qT", [D, W])
    kT = din("kT", [D, kv])
    vT = din("vT", [D, kv + 1])
    wq = din("wq", [D, D])
    wk = din("wk", [D, D])
    wv = din("wv", [D, D])
    wo = din("wo", [D, D])
    d1w = din("d1w", [D, D])
    d2w = din("d2w", [D, D])
    colneg = din("colneg", [128, kt_n], F32)
    colnegT = din("colnegT", [1, 128 * kt_n])
    wvec = din("wvec", [1, W])
    sigu = din("sigu", [1, H])
    d1b = din("d1b", [128, DT], F32) if has_d1b else None
    d2b = din("d2b", [1, D], F32) if has_d2b else None
    lng = din("lng", [1, D], F32) if has_g else None
    lnb = din("lnb", [1, D], F32) if has_b else None
    out = nc.dram_tensor("out", [L, D], F32, kind="ExternalOutput").ap()

    with tile.TileContext(nc) as tc, ExitStack() as ctx:
        # ---------------- long-lived small tiles ----------------
        plong = ctx.enter_context(tc.tile_pool(name="plong", bufs=1))
        colneg_s = plong.tile([128, kt_n], F32, name="colneg_s")
        colnegT_s = plong.tile([1, 128 * kt_n], BF, name="colnegT_s")
        wvec_s = plong.tile([1, W], BF, name="wvec_s")
        ones64 = plong.tile([1, 64], BF, name="ones64")
        nc.vector.memset(ones64[:].bitcast(U16), ONE_BF)
        vm65row = plong.tile([1, 65 * H], BF, name="vm65row")
        gb = plong.tile([128, D], F32, name="gb") if has_g else None
        bb = plong.tile([128, D], F32, name="bb") if has_b else None
        d2bb = plong.tile([128, D], F32, name="d2bb") if has_d2b else None
        epst = plong.tile([128, 1], F32, name="epst")
        d1b_s = plong.tile([128, DT], F32, name="d1b_s") if has_d1b else None

        # attnorm^T lives from attention through the Wo projection
        sBC = ExitStack()
        pbc = sBC.enter_context(tc.tile_pool(name="pbc", bufs=1))
        attnorm = [pbc.tile([128, W], BF, name=f"attnorm{j}")
                   for j in range(DT)]

        sAB = ExitStack()
        pproj = sAB.enter_context(tc.tile_pool(name="pproj", bufs=1))
        Qp = [pproj.tile([128, W], BF, name=f"Qp{j}") for j in range(DT)]
        Kp = [pproj.tile([128, kv], BF, name=f"Kp{j}") for j in range(DT)]
        Vm65 = [pproj.tile([128, 65 * H], BF, name=f"Vm65_{k}")
                for k in range(kt_n)]

        # ---------------- phase A: projections ----------------
        sA = ExitStack()
        pin = sA.enter_context(tc.tile_pool(name="pin", bufs=1))
        pw = sA.enter_context(tc.tile_pool(name="pw", bufs=3))
        psA = sA.enter_context(tc.tile_pool(name="psA", bufs=1, space="PSUM"))

        # big fused loads: [768, w] DRAM -> [128, DT*w] SBUF via 3D AP
        def big_load(dst, dram_ap, w, tsplits=((0, DT),)):
            src = dram_ap.rearrange("(t p) w -> p t w", p=128)
            dstv = dst[:].rearrange("p (t w) -> p t w", w=w)
            for (t0, t1) in tsplits:
                nc.sync.dma_start(out=dstv[:, t0:t1, :], in_=src[:, t0:t1, :])

        def xsl(big, t, w, c0, cw):
            return big[:, t * w + c0:t * w + c0 + cw]

        qTb = pin.tile([128, DT * W], BF, name="qTb")
        kTb = pin.tile([128, DT * kv], BF, name="kTb")
        vTb = pin.tile([128, DT * (kv + 1)], BF, name="vTb")
        wvb = pin.tile([128, DT * D], BF, name="wvb")

        # startup: first weight block + qT split so PE starts ~3.5us in
        wh_q0 = pw.tile([128, DT * 256], BF, tag="wh", name="wh_q0")
        big_load(wh_q0, wq[:, 0:256], 256, ((0, 3),))
        big_load(qTb, qT, W, ((0, 3),))
        big_load(wh_q0, wq[:, 0:256], 256, ((3, DT),))
        big_load(qTb, qT, W, ((3, DT),))
        nc.sync.dma_start(out=colneg_s[:], in_=colneg[:, :])
        nc.sync.dma_start(out=colnegT_s[:], in_=colnegT[:, :])
        nc.sync.dma_start(out=wvec_s[:], in_=wvec[:, :])

        # Q then K projections in transposed layout
        for (wdram, xb, xw, outts, chs, wh0) in (
                (wq, qTb, W, Qp, qch, wh_q0), (wk, kTb, kv, Kp, kch, None)):
            for jh in range(3):
                if jh == 0 and wh0 is not None:
                    wh = wh0
                else:
                    wh = pw.tile([128, DT * 256], BF, tag="wh",
                                 name=f"wh_{id(wdram) % 97}_{jh}")
                    big_load(wh, wdram[:, 256 * jh:256 * jh + 256], 256)
                if wdram is wk and jh == 0:
                    big_load(kTb, kT, kv)
                for jj in range(2):
                    j = 2 * jh + jj
                    ps = psA.tile([128, W if xw == W else kv], F32, tag="A",
                                  bufs=2, name=f"psA_{id(wdram) % 97}_{j}",
                                  padded_shape=[128, 1024])
                    for t in range(DT):
                        for (c0, cw) in chs:
                            nc.tensor.matmul(
                                ps[:, c0:c0 + cw],
                                xsl(wh, t, 256, 128 * jj, 128),
                                xsl(xb, t, xw, c0, cw),
                                start=(t == 0), stop=(t == DT - 1))
                    nc.scalar.copy(out=outts[j][:, :], in_=ps[:, :])

        big_load(vTb, vT, kv + 1)
        big_load(wvb, wv, D)
        nc.sync.dma_start(
            out=vm65row[:].rearrange("p (h e) -> p h e", e=65)[:, :, 64:65],
            in_=sigu[:, :].rearrange("p (h e) -> p h e", e=1))

        # V projection in natural layout -> Vm65 (65-stride heads) + aux row
        for i in range(vt_n):
            rows_v = min(128, kv + 1 - 128 * i)
            psv = psA.tile([rows_v, D], F32, tag="V", bufs=2, name=f"psV{i}",
                           padded_shape=[128, 768])
            for t in range(DT):
                for (n0, nw) in ((0, 512), (512, 256)):
                    nc.tensor.matmul(
                        psv[:, n0:n0 + nw],
                        xsl(vTb, t, kv + 1, 128 * i, rows_v),
                        xsl(wvb, t, D, n0, nw),
                        start=(t == 0), stop=(t == DT - 1))
            kr = max(0, min(128, kv - 128 * i))
            if kr:
                src = psv[0:kr, :].rearrange("p (h e) -> p h e", e=64)
                dst = Vm65[i][0:kr].rearrange("p (h e) -> p h e",
                                              e=65)[:, :, 0:64]
                nc.vector.tensor_copy(out=dst, in_=src)
                nc.vector.memset(
                    Vm65[i][0:kr].bitcast(U16)
                    .rearrange("p (h e) -> p h e", e=65)[:, :, 64:65], ONE_BF)
            if i == aux_t:
                # masked-v-sum @ Wv row: kv is a multiple of 32 so aux_r is a
                # legal PSUM partition base; DVE shifts it down to partition 0
                nc.vector.tensor_copy(
                    out=vm65row[:].rearrange("p (h e) -> p h e",
                                             e=65)[:, :, 0:64],
                    in_=psv[aux_r:aux_r + 1, :].rearrange(
                        "p (h e) -> p h e", e=64))

        sA.close()

        # C/D weight loads stream during phase B
        sCD = ExitStack()
        pcd = sCD.enter_context(tc.tile_pool(name="pcd", bufs=1, side="right"))
        mhaT = [pcd.tile([128, W], BF, name=f"mhaT{j}") for j in range(DT)]
        mhaN = [pcd.tile([128, D], BF, name=f"mhaN{q}") for q in range(qt_n)]
        wob = pcd.tile([128, DT * D], BF, name="wob")
        d1b_t = pcd.tile([128, DT * D], BF, name="d1b_t")
        d2b_t = pcd.tile([128, DT * D], BF, name="d2b_t")
        for (wdram, dst) in ((wo, wob), (d1w, d1b_t), (d2w, d2b_t)):
            s3 = wdram.rearrange("(t p) w -> p t w", p=128)
            d3 = dst[:].rearrange("p (t w) -> p t w", w=D)
            nc.sync.dma_start(out=d3[:, :, 0:384], in_=s3[:, :, 0:384])
            nc.sync.dma_start(out=d3[:, :, 384:768], in_=s3[:, :, 384:768])
        if has_g:
            nc.sync.dma_start(out=gb[:], in_=lng.to_broadcast([128, D]))
        if has_b:
            nc.sync.dma_start(out=bb[:], in_=lnb.to_broadcast([128, D]))
        if has_d2b:
            nc.sync.dma_start(out=d2bb[:], in_=d2b.to_broadcast([128, D]))
        if has_d1b:
            nc.sync.dma_start(out=d1b_s[:], in_=d1b[:, :])
        nc.vector.memset(epst[:], EPS)

        # ---------------- phase B: attention ----------------
        # Main query block [0:WM] in 1-bank score tiles; the narrow
        # remainder [WM:W] is batched per head into one "remband" bank
        # (scores get colneg added via a rank-1 matmul so a single bias-free
        # exp covers all key tiles).  The per-head epilogue (recip / rbp
        # broadcast / divide) is deferred by one head so its cross-engine
        # chain never head-of-line-blocks the next head's score matmuls.
        WM = min(W, 512)
        rw = W - WM
        RO_AO, RO_RB = 256, 384
        assert rw == 0 or (rw <= 64 and kt_n * rw <= RO_AO), \
            f"query remainder {rw} x {kt_n} exceeds remband layout"
        sB = ExitStack()
        ppexp = sB.enter_context(tc.tile_pool(name="ppexp", bufs=1))
        psB = sB.enter_context(tc.tile_pool(name="psB", bufs=1, space="PSUM"))

        def epilogue(h, ao, remb):
            hp, po = h // 2, 64 * (h % 2)
            rc = ppexp.tile([1, W], BF, tag="rc", bufs=2, name=f"rc{h}")
            with nc.allow_low_precision(reason="bf16 recip is ample"):
                nc.vector.reciprocal(out=rc[:, 0:WM], in_=ao[64:65, :])
                if rw:
                    nc.vector.reciprocal(
                        out=rc[:, WM:W],
                        in_=remb[64:65, RO_AO:RO_AO + rw])
            rbp = psB.tile([64, 512], F32, tag="rbp", bufs=1, name=f"rbp{h}",
                           padded_shape=[64, 512])
            nc.tensor.matmul(rbp[:, 0:WM], ones64[0:1, :], rc[0:1, 0:WM],
                             start=True, stop=True)
            if rw:
                nc.tensor.matmul(remb[0:64, RO_RB:RO_RB + rw], ones64[0:1, :],
                                 rc[0:1, WM:W], start=True, stop=True)
            rbs = ppexp.tile([64, W], BF, tag="rbs", bufs=2, name=f"rbs{h}")
            nc.vector.tensor_copy(out=rbs[:, 0:WM], in_=rbp[:, 0:WM])
            if rw:
                nc.vector.tensor_copy(out=rbs[:, WM:W],
                                      in_=remb[0:64, RO_RB:RO_RB + rw])
            nc.vector.tensor_tensor(
                out=attnorm[hp][po:po + 64, 0:WM],
                in0=ao[0:64, :], in1=rbs[:, 0:WM], op=OP.mult)
            if rw:
                nc.vector.tensor_tensor(
                    out=attnorm[hp][po:po + 64, WM:W],
                    in0=remb[0:64, RO_AO:RO_AO + rw], in1=rbs[:, WM:W],
                    op=OP.mult)

        pend = None
        for h in range(H):
            hp, po = h // 2, 64 * (h % 2)
            hs = slice(65 * h, 65 * h + 65)
            ao = psB.tile([65, WM], F32, tag="ao", bufs=2, name=f"ao{h}",
                          padded_shape=[65, 512])
            remb = psB.tile([128, 512], F32, tag="rem", bufs=2,
                            name=f"remb{h}", padded_shape=[128, 512]) \
                if rw else None
            for kt in range(kt_n):
                kr = krows(kt)
                sc = psB.tile([128, WM], F32, tag="sc", bufs=3,
                              name=f"sc{h}_{kt}", padded_shape=[128, 512])
                nc.tensor.matmul(
                    sc[0:kr, :],
                    Kp[hp][po:po + 64, 128 * kt:128 * kt + kr],
                    Qp[hp][po:po + 64, 0:WM],
                    start=True, stop=True)
                p = ppexp.tile([128, WM], BF, tag="p", bufs=3,
                               name=f"p{h}_{kt}")
                nc.scalar.activation(out=p[0:kr, :], in_=sc[0:kr, :],
                                     func=AF.Exp,
                                     bias=colneg_s[0:kr, kt:kt + 1],
                                     scale=1.0)
                nc.tensor.matmul(
                    ao[:, :], Vm65[kt][0:kr, hs], p[0:kr, :],
                    start=(kt == 0), stop=False)
            nc.tensor.matmul(ao[:, :], vm65row[0:1, hs], wvec_s[0:1, 0:WM],
                             start=False, stop=True)
            if rw:
                for kt in range(kt_n):
                    kr = krows(kt)
                    so = kt * rw
                    nc.tensor.matmul(
                        remb[0:kr, so:so + rw],
                        Kp[hp][po:po + 64, 128 * kt:128 * kt + kr],
                        Qp[hp][po:po + 64, WM:W], start=True, stop=False)
                    nc.tensor.matmul(
                        remb[0:kr, so:so + rw],
                        colnegT_s[0:1, 128 * kt:128 * kt + kr],
                        ones64[0:1, 0:rw], start=False, stop=True)
                prem = ppexp.tile([128, kt_n * rw], BF, tag="prem", bufs=2,
                                  name=f"prem{h}")
                nc.scalar.activation(out=prem[:], in_=remb[:, 0:kt_n * rw],
                                     func=AF.Exp, scale=1.0)
                for kt in range(kt_n):
                    kr = krows(kt)
                    nc.tensor.matmul(
                        remb[0:65, RO_AO:RO_AO + rw], Vm65[kt][0:kr, hs],
                        prem[0:kr, kt * rw:kt * rw + rw],
                        start=(kt == 0), stop=False)
                nc.tensor.matmul(remb[0:65, RO_AO:RO_AO + rw],
                                 vm65row[0:1, hs], wvec_s[0:1, WM:W],
                                 start=False, stop=True)
            if pend is not None:
                epilogue(*pend)
            pend = (h, ao, remb)
        epilogue(*pend)
        sB.close()
        sAB.close()

        # ---------------- phase C: Wo (transposed) + mhaN transposes ------
        qi_order = [qt_n - 1] + list(range(qt_n - 1))

        def qw_of(qi):
            return min(128, W - 128 * qi)

        from concourse.masks import make_identity
        ident = pcd.tile([128, 128], BF, name="ident")
        nc.vector.memset(ident[:].bitcast(U16), 0)
        make_identity(nc, ident[:], nomemset=True)

        sC = ExitStack()
        psC = sC.enter_context(tc.tile_pool(name="psC", bufs=1, space="PSUM"))
        for j in range(DT):
            ps = psC.tile([128, W], F32, tag="C", bufs=3, name=f"psT{j}",
                          padded_shape=[128, 1024])
            for t in range(DT):
                for (c0, cw) in qch:
                    nc.tensor.matmul(
                        ps[:, c0:c0 + cw],
                        wob[:, t * D + 128 * j:t * D + 128 * j + 128],
                        attnorm[t][:, c0:c0 + cw],
                        start=(t == 0), stop=(t == DT - 1))
            nc.scalar.copy(out=mhaT[j][:, :], in_=ps[:, :])

        # mhaN via PE transposes (collect all 6 blocks in one bf16 psum tile)
        for qi in qi_order:
            qw = qw_of(qi)
            coll = psC.tile([128, D], BF, tag="T", bufs=2, name=f"coll{qi}",
                            padded_shape=[128, 768])
            for j in range(DT):
                nc.tensor.transpose(
                    coll[0:qw, 128 * j:128 * j + 128],
                    mhaT[j][:, 128 * qi:128 * qi + qw], ident[:])
            if has_d2b:
                nc.vector.scalar_tensor_tensor(
                    out=mhaN[qi][0:qw, :], in0=coll[0:qw, :], scalar=0.0,
                    in1=d2bb[0:qw, :], op0=OP.bypass, op1=OP.add)
            else:
                nc.vector.tensor_copy(out=mhaN[qi][0:qw, :],
                                      in_=coll[0:qw, :])
        sC.close()
        sBC.close()

        # ---------------- phase D: FFN + LayerNorm ----------------
        sD = ExitStack()
        pdx = sD.enter_context(tc.tile_pool(name="pdx", bufs=1, side="right"))
        psmall = sD.enter_context(
            tc.tile_pool(name="psmall", bufs=8, side="right"))
        psD = sD.enter_context(tc.tile_pool(name="psD", bufs=1, space="PSUM"))

        reluT = [pdx.tile([128, W], BF, name=f"reluT{j}") for j in range(DT)]
        for j in range(DT):
            ps = psD.tile([128, W], F32, tag="D", bufs=2, name=f"psd1_{j}",
                          padded_shape=[128, 1024])
            for t in range(DT):
                for (c0, cw) in qch:
                    nc.tensor.matmul(
                        ps[:, c0:c0 + cw],
                        d1b_t[:, t * D + 128 * j:t * D + 128 * j + 128],
                        mhaT[t][:, c0:c0 + cw],
                        start=(t == 0), stop=(t == DT - 1))
            if has_d1b:
                nc.scalar.activation(out=reluT[j][:, :], in_=ps[:, :],
                                     func=AF.Relu, bias=d1b_s[:, j:j + 1],
                                     scale=1.0)
            else:
                nc.scalar.activation(out=reluT[j][:, :], in_=ps[:, :],
                                     func=AF.Relu, scale=1.0)

        inv_d = 1.0 / D

        # stage 1 per token tile: d2 matmul, residual add (+row sums),
        # squares.  stage 2: the LN scalar chain + normalize + store.
        # Emitting all stage-1 blocks first lets the per-tile LN chains
        # overlap across tiles instead of serializing the tail.
        st1 = {}
        for qi in qi_order:
            qw = qw_of(qi)
            ps = psD.tile([qw, D], F32, tag="D2", bufs=2, name=f"psff{qi}",
                          padded_shape=[128, 768])
            for (n0, nw) in ((0, 512), (512, 256)):
                for t in range(DT):
                    nc.tensor.matmul(
                        ps[:, n0:n0 + nw],
                        reluT[t][:, 128 * qi:128 * qi + qw],
                        d2b_t[:, t * D + n0:t * D + n0 + nw],
                        start=(t == 0), stop=(t == DT - 1))
            x = pdx.tile([qw, D], F32, tag="x", bufs=qt_n, name=f"x{qi}")
            xsum = psmall.tile([qw, 1], F32, tag="s1", name=f"xsum{qi}")
            nc.vector.scalar_tensor_tensor(out=x[:], in0=ps[:, :], scalar=0.0,
                                           in1=mhaN[qi][0:qw, :],
                                           op0=OP.bypass, op1=OP.add,
                                           accum_out=xsum[:])
            scr = pdx.tile([qw, D], F32, tag="scr", bufs=qt_n,
                           name=f"scr{qi}")
            xsq = psmall.tile([qw, 1], F32, tag="s2", name=f"xsq{qi}")
            nc.scalar.activation(out=scr[:], in_=x[:], func=AF.Square,
                                 accum_out=xsq[:])
            mu = psmall.tile([qw, 1], F32, tag="s3", name=f"mu{qi}")
            nc.vector.tensor_scalar_mul(out=mu[:], in0=xsum[:],
                                        scalar1=inv_d)
            mu2 = psmall.tile([qw, 1], F32, tag="s5", name=f"mu2{qi}")
            nc.gpsimd.tensor_tensor(out=mu2[:], in0=mu[:], in1=mu[:],
                                    op=OP.mult)
            st1[qi] = (qw, x, scr, xsq, mu, mu2)

        for qi in qi_order:
            qw, x, scr, xsq, mu, mu2 = st1[qi]
            var = psmall.tile([qw, 1], F32, tag="s4", name=f"var{qi}")
            nc.vector.scalar_tensor_tensor(out=var[:], in0=xsq[:],
                                           scalar=inv_d, in1=mu2[:],
                                           op0=OP.mult, op1=OP.subtract)
            std = psmall.tile([qw, 1], F32, tag="s6", name=f"std{qi}")
            nc.scalar.activation(out=std[:], in_=var[:], func=AF.Sqrt,
                                 bias=epst[0:qw, :], scale=1.0)
            rstd = psmall.tile([qw, 1], F32, tag="s7", name=f"rstd{qi}")
            nc.vector.reciprocal(out=rstd[:], in_=std[:])
            nmb = psmall.tile([qw, 1], F32, tag="s8", name=f"nmb{qi}")
            nc.vector.scalar_tensor_tensor(out=nmb[:], in0=mu[:], scalar=-1.0,
                                           in1=rstd[:], op0=OP.mult,
                                           op1=OP.mult)
            cur = scr
            if qi % 2:
                nc.scalar.activation(out=cur[:], in_=x[:], func=AF.Identity,
                                     bias=nmb[:], scale=rstd[:])
            else:
                nc.vector.tensor_scalar(out=cur[:], in0=x[:],
                                        scalar1=rstd[:], scalar2=nmb[:],
                                        op0=OP.mult, op1=OP.add)
            if has_g:
                nc.vector.tensor_tensor(out=x[:], in0=cur[:], in1=gb[0:qw, :],
                                        op=OP.mult)
                cur = x
            if has_b:
                xo = pdx.tile([qw, D], F32, tag="xo", bufs=2, name=f"xo{qi}")
                nc.gpsimd.tensor_tensor(out=xo[:], in0=cur[:], in1=bb[0:qw, :],
                                        op=OP.add)
                cur = xo
            nc.sync.dma_start(out=out[128 * qi:128 * qi + qw, :], in_=cur[:])
            if qi == qt_n - 1 and W < L:
                # rows W..L all equal row W-1 (shared masked-row value).
                # DRAM->DRAM broadcast; same-queue FIFO orders it after the
                # write of row W-1 just above.
                nc.sync.dma_start(
                    out=out[W:L, :],
                    in_=out[W - 1:W, :].to_broadcast([L - W, D]))
        sD.close()
        sCD.close()

    nc.compile()
    return nc


_PROGRAM_CACHE = {}


def _get_program(kv, qv, n_cores, has_d1b, has_d2b, has_g, has_b):
    key = (kv, qv, n_cores, has_d1b, has_d2b, has_g, has_b)
    if key not in _PROGRAM_CACHE:
        _PROGRAM_CACHE[key] = build_program(kv, qv, n_cores, has_d1b,
                                            has_d2b, has_g, has_b)
    return _PROGRAM_CACHE[key]


def make_in_map(b, kv, qv, flags, queries, keys, values, mask_1, mask_2,
                Wq, Wk, Wv, Wo, d1_w, d1_b, d2_w, d2_b, ln_g, ln_b):
    has_d1b, has_d2b, has_g, has_b = flags
    W = min(qv + 1, L)
    kt_n = (kv + 127) // 128
    f32, bf16 = np.float32, ml_dtypes.bfloat16
    vl1 = int(np.count_nonzero(mask_1[b]))
    vl2 = int(np.count_nonzero(mask_2[b]))
    row01 = (np.arange(L) < vl2).astype(f32)
    qmT = (np.asarray(queries[b], f32) * row01[:, None]).T
    vb = np.asarray(values[b], f32)
    vaux_m = vb[vl1:, :].sum(0, dtype=np.float64).astype(f32)
    vTf = np.concatenate([vb.T[:, :kv], vaux_m[:, None]], axis=1)
    cnp = np.arange(128 * kt_n).reshape(kt_n, 128).T
    cn = np.where(cnp < vl1, 0.0, NEG).astype(f32)
    m = {
        "qT": np.ascontiguousarray(qmT[:, :W]).astype(bf16),
        "kT": np.ascontiguousarray(np.asarray(keys[b], f32).T[:, :kv]
                                   ).astype(bf16),
        "vT": np.ascontiguousarray(vTf).astype(bf16),
        "wq": (np.asarray(Wq, f32) * 0.125).astype(bf16),
        "wk": np.asarray(Wk, f32).astype(bf16),
        "wv": np.asarray(Wv, f32).astype(bf16),
        "wo": np.asarray(Wo, f32).astype(bf16),
        "d1w": np.asarray(d1_w, f32).astype(bf16),
        "d2w": np.asarray(d2_w, f32).astype(bf16),
        "colneg": np.ascontiguousarray(cn),
        "colnegT": np.where(np.arange(128 * kt_n) < vl1, 0.0,
                            NEG)[None, :].astype(bf16),
        "wvec": (1.0 - row01)[None, :W].astype(bf16),
        "sigu": np.full((1, H), float(L - vl1), f32).astype(bf16),
    }
    if has_d1b:
        m["d1b"] = np.ascontiguousarray(
            np.asarray(d1_b, f32).reshape(DT, 128).T)
    if has_d2b:
        m["d2b"] = np.asarray(d2_b, f32)[None, :]
    if has_g:
        m["lng"] = np.asarray(ln_g, f32)[None, :]
    if has_b:
        m["lnb"] = np.asarray(ln_b, f32)[None, :]
    return m


def kernel(queries, keys, values, mask_1, mask_2,
           Wq, Wk, Wv, Wo, d1_w, d1_b, d2_w, d2_b, ln_g, ln_b):
    from concourse.bass_utils import run_bass_kernel_spmd

    queries = np.asarray(queries)
    B = queries.shape[0]
    vl1 = np.count_nonzero(np.asarray(mask_1), axis=1)
    vl2 = np.count_nonzero(np.asarray(mask_2), axis=1)
    kv = int(((max(vl1.max(), 256) + 31) // 32) * 32)
    qv = int(max(vl2.max(), 256))
    flags = (bool(np.any(np.asarray(d1_b))), bool(np.any(np.asarray(d2_b))),
             not np.all(np.asarray(ln_g) == 1.0),
             bool(np.any(np.asarray(ln_b))))
    nc = _get_program(kv, qv, B, *flags)
    in_maps = [
        make_in_map(b, kv, qv, flags, queries, keys, values, mask_1, mask_2,
                    Wq, Wk, Wv, Wo, d1_w, d1_b, d2_w, d2_b, ln_g, ln_b)
        for b in range(B)
    ]
    res = run_bass_kernel_spmd(nc, in_maps, list(range(B)))
    return np.stack([res.results[b]["out"] for b in range(B)], axis=0)


# revision 19
# speedup vs baseline: 1.0008x; 1.0008x over previous
"""Trainium2 Bass kernel for a cross-attention transformer block.

Contract: kernel(**inputs) takes the FULL inputs (B=8 batch), shards
batch-wise across 8 NeuronCores (one batch element per core, SPMD, no
collectives), runs a Bass/Tile kernel, and returns the FULL output.

v2 design (vs the 177µs fp32r baseline):
  - all GEMM operands bf16 (1 cyc/row on PE at any width, half the DMA
    bytes, ~4.5e-3 end-to-end rel err vs the 2e-2 budget)
  - exact valid lengths: keys kv = max(vl1), query width W = max(vl2)+1
    (no padding to 128); the extra query column is an all-zero query with
    wvec=1, which makes attention produce the shared "masked row" value
    (mean over all 1024 keys of v@Wv) in column W-1
  - phases C/D (Wo, FFN, LayerNorm) run on W tokens instead of 1024;
    output rows W..1024 are a broadcast-DMA of row W-1
  - softmax exp: one ACT instruction per (head, key-tile) over the full
    [rows, W] contiguous PSUM region (bias = per-partition colneg)
  - masked-key aux row (sum of invalid v rows) rides along as an extra
    column of vT through the V projection; extracted from PSUM by DMA
  - per-head pipeline: scores (PE) -> exp (ACT) -> attn@V+denominator
    (PE, M=65 with a ones row) -> rank-1 wvec correction (PE) ->
    reciprocal + broadcast-by-matmul + divide (DVE/PE)
"""

import sys

for _p in ("/opt/trn_rl_repo",):
    if _p not in sys.path:
        sys.path.insert(0, _p)

from contextlib import ExitStack

import numpy as np
import ml_dtypes

import concourse.bacc as bacc
import concourse.tile as tile
from concourse import mybir

F32 = mybir.dt.float32
BF = mybir.dt.bfloat16
U16 = mybir.dt.uint16
AF = mybir.ActivationFunctionType
OP = mybir.AluOpType

D = 768
H = 12
HD = 64
DT = 6          # feature tiles of 128
L = 1024
NEG = -1000000.0
EPS = 1e-5
ONE_BF = 0x3F80  # 1.0 in bfloat16 bits


def _chunks(w):
    """Split width into PSUM-bank-sized matmul chunks (<=512 each)."""
    out, off = [], 0
    while w > 0:
        c = min(512, w)
        out.append((off, c))
        off += c
        w -= c
    return out


def build_program(kv, qv, n_cores, has_d1b=False, has_d2b=False,
                  has_g=False, has_b=False):
    W = min(qv + 1, L)          # query width incl tail column
    qch = _chunks(W)
    kch = _chunks(kv)
    kt_n = (kv + 127) // 128
    vt_n = (kv + 1 + 127) // 128  # vT has kv+1 cols (aux masked-sum)
    aux_t, aux_r = kv // 128, kv % 128
    qt_n = (W + 127) // 128       # token tiles for natural-layout phases

    def krows(i):
        return min(128, kv - 128 * i)

    nc = bacc.Bacc("TRN2", target_bir_lowering=False, debug=False,
                   num_devices=n_cores)

    def din(name, shape, dt=BF):
        return nc.dram_tensor(name, shape, dt, kind="ExternalInput").ap()

    qT = din("qT", [D, W])
    kT = din("kT", [D, kv])
    vT = din("vT", [D, kv + 1])
    wq = din("wq", [D, D])
    wk = din("wk", [D, D])
    wv = din("wv", [D, D])
    wo = din("wo", [D, D])
    d1w = din("d1w", [D, D])
    d2w = din("d2w", [D, D])
    colneg = din("colneg", [128, kt_n], F32)
    colnegT = din("colnegT", [1, 128 * kt_n])
    wvec = din("wvec", [1, W])
    sigu = din("sigu", [1, H])
    d1b = din("d1b", [128, DT], F32) if has_d1b else None
    d2b = din("d2b", [1, D], F32) if has_d2b else None
    lng = din("lng", [1, D], F32) if has_g else None
    lnb = din("lnb", [1, D], F32) if has_b else None
    out = nc.dram_tensor("out", [L, D], F32, kind="ExternalOutput").ap()

    with tile.TileContext(nc) as tc, ExitStack() as ctx:
        # ---------------- long-lived small tiles ----------------
        plong = ctx.enter_context(tc.tile_pool(name="plong", bufs=1))
        colneg_s = plong.tile([128, kt_n], F32, name="colneg_s")
        colnegT_s = plong.tile([1, 128 * kt_n], BF, name="colnegT_s")
        wvec_s = plong.tile([1, W], BF, name="wvec_s")
        ones64 = plong.tile([1, 64], BF, name="ones64")
        nc.vector.memset(ones64[:].bitcast(U16), ONE_BF)
        vm65row = plong.tile([1, 65 * H], BF, name="vm65row")
        gb = plong.tile([128, D], F32, name="gb") if has_g else None
        bb = plong.tile([128, D], F32, name="bb") if has_b else None
        d2bb = plong.tile([128, D], F32, name="d2bb") if has_d2b else None
        epst = plong.tile([128, 1], F32, name="epst")
        d1b_s = plong.tile([128, DT], F32, name="d1b_s") if has_d1b else None

        # attnorm^T lives from attention through the Wo projection
        sBC = ExitStack()
        pbc = sBC.enter_context(tc.tile_pool(name="pbc", bufs=1))
        attnorm = [pbc.tile([128, W], BF, name=f"attnorm{j}")
                   for j in range(DT)]

        sAB = ExitStack()
        pproj = sAB.enter_context(tc.tile_pool(name="pproj", bufs=1))
        Qp = [pproj.tile([128, W], BF, name=f"Qp{j}") for j in range(DT)]
        Kp = [pproj.tile([128, kv], BF, name=f"Kp{j}") for j in range(DT)]
        Vm65 = [pproj.tile([128, 65 * H], BF, name=f"Vm65_{k}")
                for k in range(kt_n)]

        # ---------------- phase A: projections ----------------
        sA = ExitStack()
        pin = sA.enter_context(tc.tile_pool(name="pin", bufs=1))
        pw = sA.enter_context(tc.tile_pool(name="pw", bufs=3))
        psA = sA.enter_context(tc.tile_pool(name="psA", bufs=1, space="PSUM"))

        # big fused loads: [768, w] DRAM -> [128, DT*w] SBUF via 3D AP
        def big_load(dst, dram_ap, w, tsplits=((0, DT),)):
            src = dram_ap.rearrange("(t p) w -> p t w", p=128)
            dstv = dst[:].rearrange("p (t w) -> p t w", w=w)
            for (t0, t1) in tsplits:
                nc.sync.dma_start(out=dstv[:, t0:t1, :], in_=src[:, t0:t1, :])

        def xsl(big, t, w, c0, cw):
            return big[:, t * w + c0:t * w + c0 + cw]

        qTb = pin.tile([128, DT * W], BF, name="qTb")
        kTb = pin.tile([128, DT * kv], BF, name="kTb")
        vTb = pin.tile([128, DT * (kv + 1)], BF, name="vTb")
        wvb = pin.tile([128, DT * D], BF, name="wvb")

        # startup: first weight block + qT split so PE starts ~3.5us in
        wh_q0 = pw.tile([128, DT * 256], BF, tag="wh", name="wh_q0")
        big_load(wh_q0, wq[:, 0:256], 256, ((0, 3),))
        big_load(qTb, qT, W, ((0, 3),))
        big_load(wh_q0, wq[:, 0:256], 256, ((3, DT),))
        big_load(qTb, qT, W, ((3, DT),))
        nc.sync.dma_start(out=colneg_s[:], in_=colneg[:, :])
        nc.sync.dma_start(out=colnegT_s[:], in_=colnegT[:, :])
        nc.sync.dma_start(out=wvec_s[:], in_=wvec[:, :])

        # Q then K projections in transposed layout
        for (wdram, xb, xw, outts, chs, wh0) in (
                (wq, qTb, W, Qp, qch, wh_q0), (wk, kTb, kv, Kp, kch, None)):
            for jh in range(3):
                if jh == 0 and wh0 is not None:
                    wh = wh0
                else:
                    wh = pw.tile([128, DT * 256], BF, tag="wh",
                                 name=f"wh_{id(wdram) % 97}_{jh}")
                    big_load(wh, wdram[:, 256 * jh:256 * jh + 256], 256)
                if wdram is wk and jh == 0:
                    big_load(kTb, kT, kv)
                for jj in range(2):
                    j = 2 * jh + jj
                    ps = psA.tile([128, W if xw == W else kv], F32, tag="A",
                                  bufs=2, name=f"psA_{id(wdram) % 97}_{j}",
                                  padded_shape=[128, 1024])
                    for t in range(DT):
                        for (c0, cw) in chs:
                            nc.tensor.matmul(
                                ps[:, c0:c0 + cw],
                                xsl(wh, t, 256, 128 * jj, 128),
                                xsl(xb, t, xw, c0, cw),
                                start=(t == 0), stop=(t == DT - 1))
                    nc.scalar.copy(out=outts[j][:, :], in_=ps[:, :])

        big_load(vTb, vT, kv + 1)
        big_load(wvb, wv, D)
        nc.sync.dma_start(
            out=vm65row[:].rearrange("p (h e) -> p h e", e=65)[:, :, 64:65],
            in_=sigu[:, :].rearrange("p (h e) -> p h e", e=1))

        # V projection in natural layout -> Vm65 (65-stride heads) + aux row
        for i in range(vt_n):
            rows_v = min(128, kv + 1 - 128 * i)
            psv = psA.tile([rows_v, D], F32, tag="V", bufs=2, name=f"psV{i}",
                           padded_shape=[128, 768])
            for t in range(DT):
                for (n0, nw) in ((0, 512), (512, 256)):
                    nc.tensor.matmul(
                        psv[:, n0:n0 + nw],
                        xsl(vTb, t, kv + 1, 128 * i, rows_v),
                        xsl(wvb, t, D, n0, nw),
                        start=(t == 0), stop=(t == DT - 1))
            kr = max(0, min(128, kv - 128 * i))
            if kr:
                src = psv[0:kr, :].rearrange("p (h e) -> p h e", e=64)
                dst = Vm65[i][0:kr].rearrange("p (h e) -> p h e",
                                              e=65)[:, :, 0:64]
                nc.vector.tensor_copy(out=dst, in_=src)
                nc.vector.memset(
                    Vm65[i][0:kr].bitcast(U16)
                    .rearrange("p (h e) -> p h e", e=65)[:, :, 64:65], ONE_BF)
            if i == aux_t:
                # masked-v-sum @ Wv row: kv is a multiple of 32 so aux_r is a
                # legal PSUM partition base; DVE shifts it down to partition 0
                nc.vector.tensor_copy(
                    out=vm65row[:].rearrange("p (h e) -> p h e",
                                             e=65)[:, :, 0:64],
                    in_=psv[aux_r:aux_r + 1, :].rearrange(
                        "p (h e) -> p h e", e=64))

        sA.close()

        # C/D weight loads stream during phase B
        sCD = ExitStack()
        pcd = sCD.enter_context(tc.tile_pool(name="pcd", bufs=1, side="right"))
        mhaT = [pcd.tile([128, W], BF, name=f"mhaT{j}") for j in range(DT)]
        mhaN = [pcd.tile([128, D], BF, name=f"mhaN{q}") for q in range(qt_n)]
        wob = pcd.tile([128, DT * D], BF, name="wob")
        d1b_t = pcd.tile([128, DT * D], BF, name="d1b_t")
        d2b_t = pcd.tile([128, DT * D], BF, name="d2b_t")
        for (wdram, dst) in ((wo, wob), (d1w, d1b_t), (d2w, d2b_t)):
            s3 = wdram.rearrange("(t p) w -> p t w", p=128)
            d3 = dst[:].rearrange("p (t w) -> p t w", w=D)
            nc.sync.dma_start(out=d3[:, :, 0:384], in_=s3[:, :, 0:384])
            nc.sync.dma_start(out=d3[:, :, 384:768], in_=s3[:, :, 384:768])
        if has_g:
            nc.sync.dma_start(out=gb[:], in_=lng.to_broadcast([128, D]))
        if has_b:
            nc.sync.dma_start(out=bb[:], in_=lnb.to_broadcast([128, D]))
        if has_d2b:
            nc.sync.dma_start(out=d2bb[:], in_=d2b.to_broadcast([128, D]))
        if has_d1b:
            nc.sync.dma_start(out=d1b_s[:], in_=d1b[:, :])
        nc.vector.memset(epst[:], EPS)

        # ---------------- phase B: attention ----------------
        # Main query block [0:WM] in 1-bank score tiles; the narrow
        # remainder [WM:W] is batched per head into one "remband" bank
        # (scores get colneg added via a rank-1 matmul so a single bias-free
        # exp covers all key tiles).  The per-head epilogue (recip / rbp
        # broadcast / divide) is deferred by one head so its cross-engine
        # chain never head-of-line-blocks the next head's score matmuls.
        WM = min(W, 512)
        rw = W - WM
        RO_AO, RO_RB = 256, 384
        assert rw == 0 or (rw <= 64 and kt_n * rw <= RO_AO), \
            f"query remainder {rw} x {kt_n} exceeds remband layout"
        sB = ExitStack()
        ppexp = sB.enter_context(tc.tile_pool(name="ppexp", bufs=1))
        psB = sB.enter_context(tc.tile_pool(name="psB", bufs=1, space="PSUM"))

        def epilogue(h, ao, remb):
            hp, po = h // 2, 64 * (h % 2)
            rc = ppexp.tile([1, W], BF, tag="rc", bufs=2, name=f"rc{h}")
            with nc.allow_low_precision(reason="bf16 recip is ample"):
                nc.vector.reciprocal(out=rc[:, 0:WM], in_=ao[64:65, :])
                if rw:
                    nc.vector.reciprocal(
                        out=rc[:, WM:W],
                        in_=remb[64:65, RO_AO:RO_AO + rw])
            rbp = psB.tile([64, 512], F32, tag="rbp", bufs=1, name=f"rbp{h}",
                           padded_shape=[64, 512])
            nc.tensor.matmul(rbp[:, 0:WM], ones64[0:1, :], rc[0:1, 0:WM],
                             start=True, stop=True)
            if rw:
                nc.tensor.matmul(remb[0:64, RO_RB:RO_RB + rw], ones64[0:1, :],
                                 rc[0:1, WM:W], start=True, stop=True)
            rbs = ppexp.tile([64, W], BF, tag="rbs", bufs=2, name=f"rbs{h}")
            nc.vector.tensor_copy(out=rbs[:, 0:WM], in_=rbp[:, 0:WM])
            if rw:
                nc.vector.tensor_copy(out=rbs[:, WM:W],
                                      in_=remb[0:64, RO_RB:RO_RB + rw])
            nc.vector.tensor_tensor(
                out=attnorm[hp][po:po + 64, 0:WM],
                in0=ao[0:64, :], in1=rbs[:, 0:WM], op=OP.mult)
            if rw:
                nc.vector.tensor_tensor(
                    out=attnorm[hp][po:po + 64, WM:W],
                    in0=remb[0:64, RO_AO:RO_AO + rw], in1=rbs[:, WM:W],
                    op=OP.mult)

        pend = None
        for h in range(H):
            hp, po = h // 2, 64 * (h % 2)
            hs = slice(65 * h, 65 * h + 65)
            ao = psB.tile([65, WM], F32, tag="ao", bufs=2, name=f"ao{h}",
                          padded_shape=[65, 512])
            remb = psB.tile([128, 512], F32, tag="rem", bufs=2,
                            name=f"remb{h}", padded_shape=[128, 512]) \
                if rw else None
            for kt in range(kt_n):
                kr = krows(kt)
                sc = psB.tile([128, WM], F32, tag="sc", bufs=3,
                              name=f"sc{h}_{kt}", padded_shape=[128, 512])
                nc.tensor.matmul(
                    sc[0:kr, :],
                    Kp[hp][po:po + 64, 128 * kt:128 * kt + kr],
                    Qp[hp][po:po + 64, 0:WM],
                    start=True, stop=True)
                p = ppexp.tile([128, WM], BF, tag="p", bufs=3,
                               name=f"p{h}_{kt}")
                nc.scalar.activation(out=p[0:kr, :], in_=sc[0:kr, :],
                                     func=AF.Exp,
                                     bias=colneg_s[0:kr, kt:kt + 1],
                                     scale=1.0)
                nc.tensor.matmul(
                    ao[:, :], Vm65[kt][0:kr, hs], p[0:kr, :],
                    start=(kt == 0), stop=False)
            nc.tensor.matmul(ao[:, :], vm65row[0:1, hs], wvec_s[0:1, 0:WM],
                             start=False, stop=True)
            if rw:
                for kt in range(kt_n):
                    kr = krows(kt)
                    so = kt * rw
                    nc.tensor.matmul(
                        remb[0:kr, so:so + rw],
                        Kp[hp][po:po + 64, 128 * kt:128 * kt + kr],
                        Qp[hp][po:po + 64, WM:W], start=True, stop=False)
                    nc.tensor.matmul(
                        remb[0:kr, so:so + rw],
                        colnegT_s[0:1, 128 * kt:128 * kt + kr],
                        ones64[0:1, 0:rw], start=False, stop=True)
                prem = ppexp.tile([128, kt_n * rw], BF, tag="prem", bufs=2,
                                  name=f"prem{h}")
                nc.scalar.activation(out=prem[:], in_=remb[:, 0:kt_n * rw],
                                     func=AF.Exp, scale=1.0)
                for kt in range(kt_n):
                    kr = krows(kt)
                    nc.tensor.matmul(
                        remb[0:65, RO_AO:RO_AO + rw], Vm65[kt][0:kr, hs],
                        prem[0:kr, kt * rw:kt * rw + rw],
                        start=(kt == 0), stop=False)
                nc.tensor.matmul(remb[0:65, RO_AO:RO_AO + rw],
                                 vm65row[0:1, hs], wvec_s[0:1, WM:W],
                                 start=False, stop=True)
            if pend is not None:
                epilogue(*pend)
            pend = (h, ao, remb)
        epilogue(*pend)
        sB.close()
        sAB.close()

        # ---------------- phase C: Wo (transposed) + mhaN transposes ------
        qi_order = [qt_n - 1] + list(range(qt_n - 1))

        def qw_of(qi):
            return min(128, W - 128 * qi)

        from concourse.masks import make_identity
        ident = pcd.tile([128, 128], BF, name="ident")
        nc.vector.memset(ident[:].bitcast(U16), 0)
        make_identity(nc, ident[:], nomemset=True)

        sC = ExitStack()
        psC = sC.enter_context(tc.tile_pool(name="psC", bufs=1, space="PSUM"))
        for j in range(DT):
            ps = psC.tile([128, W], F32, tag="C", bufs=3, name=f"psT{j}",
                          padded_shape=[128, 1024])
            for t in range(DT):
                for (c0, cw) in qch:
                    nc.tensor.matmul(
                        ps[:, c0:c0 + cw],
                        wob[:, t * D + 128 * j:t * D + 128 * j + 128],
                        attnorm[t][:, c0:c0 + cw],
                        start=(t == 0), stop=(t == DT - 1))
            nc.scalar.copy(out=mhaT[j][:, :], in_=ps[:, :])

        # mhaN via PE transposes (collect all 6 blocks in one bf16 psum tile)
        for qi in qi_order:
            qw = qw_of(qi)
            coll = psC.tile([128, D], BF, tag="T", bufs=2, name=f"coll{qi}",
                            padded_shape=[128, 768])
            for j in range(DT):
                nc.tensor.transpose(
                    coll[0:qw, 128 * j:128 * j + 128],
                    mhaT[j][:, 128 * qi:128 * qi + qw], ident[:])
            if has_d2b:
                nc.vector.scalar_tensor_tensor(
                    out=mhaN[qi][0:qw, :], in0=coll[0:qw, :], scalar=0.0,
                    in1=d2bb[0:qw, :], op0=OP.bypass, op1=OP.add)
            else:
                nc.vector.tensor_copy(out=mhaN[qi][0:qw, :],
                                      in_=coll[0:qw, :])
        sC.close()
        sBC.close()

        # ---------------- phase D: FFN + LayerNorm ----------------
        sD = ExitStack()
        pdx = sD.enter_context(tc.tile_pool(name="pdx", bufs=1, side="right"))
        psmall = sD.enter_context(
            tc.tile_pool(name="psmall", bufs=8, side="right"))
        psD = sD.enter_context(tc.tile_pool(name="psD", bufs=1, space="PSUM"))

        reluT = [pdx.tile([128, W], BF, name=f"reluT{j}") for j in range(DT)]
        for j in range(DT):
            ps = psD.tile([128, W], F32, tag="D", bufs=2, name=f"psd1_{j}",
                          padded_shape=[128, 1024])
            for t in range(DT):
                for (c0, cw) in qch:
                    nc.tensor.matmul(
                        ps[:, c0:c0 + cw],
                        d1b_t[:, t * D + 128 * j:t * D + 128 * j + 128],
                        mhaT[t][:, c0:c0 + cw],
                        start=(t == 0), stop=(t == DT - 1))
            if has_d1b:
                nc.scalar.activation(out=reluT[j][:, :], in_=ps[:, :],
                                     func=AF.Relu, bias=d1b_s[:, j:j + 1],
                                     scale=1.0)
            else:
                nc.scalar.activation(out=reluT[j][:, :], in_=ps[:, :],
                                     func=AF.Relu, scale=1.0)

        inv_d = 1.0 / D

        # stage 1 per token tile: d2 matmul, residual add (+row sums),
        # squares.  stage 2: the LN scalar chain + normalize + store.
        # Emitting all stage-1 blocks first lets the per-tile LN chains
        # overlap across tiles instead of serializing the tail.
        st1 = {}

        def stage1(qi):
            qw = qw_of(qi)
            ps = psD.tile([qw, D], F32, tag="D2", bufs=2, name=f"psff{qi}",
                          padded_shape=[128, 768])
            for (n0, nw) in ((0, 512), (512, 256)):
                for t in range(DT):
                    nc.tensor.matmul(
                        ps[:, n0:n0 + nw],
                        reluT[t][:, 128 * qi:128 * qi + qw],
                        d2b_t[:, t * D + n0:t * D + n0 + nw],
                        start=(t == 0), stop=(t == DT - 1))
            x = pdx.tile([qw, D], F32, tag="x", bufs=qt_n, name=f"x{qi}")
            xsum = psmall.tile([qw, 1], F32, tag="s1", name=f"xsum{qi}")
            nc.vector.scalar_tensor_tensor(out=x[:], in0=ps[:, :], scalar=0.0,
                                           in1=mhaN[qi][0:qw, :],
                                           op0=OP.bypass, op1=OP.add,
                                           accum_out=xsum[:])
            scr = pdx.tile([qw, D], F32, tag="scr", bufs=qt_n,
                           name=f"scr{qi}")
            xsq = psmall.tile([qw, 1], F32, tag="s2", name=f"xsq{qi}")
            nc.scalar.activation(out=scr[:], in_=x[:], func=AF.Square,
                                 accum_out=xsq[:])
            mu = psmall.tile([qw, 1], F32, tag="s3", name=f"mu{qi}")
            nc.vector.tensor_scalar_mul(out=mu[:], in0=xsum[:],
                                        scalar1=inv_d)
            mu2 = psmall.tile([qw, 1], F32, tag="s5", name=f"mu2{qi}")
            nc.gpsimd.tensor_tensor(out=mu2[:], in0=mu[:], in1=mu[:],
                                    op=OP.mult)
            st1[qi] = (qw, x, scr, xsq, mu, mu2)

        def stage2(qi):
            qw, x, scr, xsq, mu, mu2 = st1[qi]
            var = psmall.tile([qw, 1], F32, tag="s4", name=f"var{qi}")
            nc.vector.scalar_tensor_tensor(out=var[:], in0=xsq[:],
                                           scalar=inv_d, in1=mu2[:],
                                           op0=OP.mult, op1=OP.subtract)
            std = psmall.tile([qw, 1], F32, tag="s6", name=f"std{qi}")
            nc.scalar.activation(out=std[:], in_=var[:], func=AF.Sqrt,
                                 bias=epst[0:qw, :], scale=1.0)
            rstd = psmall.tile([qw, 1], F32, tag="s7", name=f"rstd{qi}")
            nc.vector.reciprocal(out=rstd[:], in_=std[:])
            nmb = psmall.tile([qw, 1], F32, tag="s8", name=f"nmb{qi}")
            nc.vector.scalar_tensor_tensor(out=nmb[:], in0=mu[:], scalar=-1.0,
                                           in1=rstd[:], op0=OP.mult,
                                           op1=OP.mult)
            cur = scr
            if qi % 2:
                nc.scalar.activation(out=cur[:], in_=x[:], func=AF.Identity,
                                     bias=nmb[:], scale=rstd[:])
            else:
                nc.vector.tensor_scalar(out=cur[:], in0=x[:],
                                        scalar1=rstd[:], scalar2=nmb[:],
                                        op0=OP.mult, op1=OP.add)
            if has_g:
                nc.vector.tensor_tensor(out=x[:], in0=cur[:], in1=gb[0:qw, :],
                                        op=OP.mult)
                cur = x
            if has_b:
                xo = pdx.tile([qw, D], F32, tag="xo", bufs=2, name=f"xo{qi}")
                nc.gpsimd.tensor_tensor(out=xo[:], in0=cur[:], in1=bb[0:qw, :],
                                        op=OP.add)
                cur = xo
            nc.sync.dma_start(out=out[128 * qi:128 * qi + qw, :], in_=cur[:])
            if qi == qt_n - 1 and W < L:
                # rows W..L all equal row W-1 (shared masked-row value).
                # DRAM->DRAM broadcast; same-queue FIFO orders it after the
                # write of row W-1 just above.
                nc.sync.dma_start(
                    out=out[W:L, :],
                    in_=out[W - 1:W, :].to_broadcast([L - W, D]))

        prev = None
        for qi in qi_order:
            stage1(qi)
            if prev is not None:
                stage2(prev)
            prev = qi
        stage2(prev)
        sD.close()
        sCD.close()

    nc.compile()
    return nc


_PROGRAM_CACHE = {}


def _get_program(kv, qv, n_cores, has_d1b, has_d2b, has_g, has_b):
    key = (kv, qv, n_cores, has_d1b, has_d2b, has_g, has_b)
    if key not in _PROGRAM_CACHE:
        _PROGRAM_CACHE[key] = build_program(kv, qv, n_cores, has_d1b,
                                            has_d2b, has_g, has_b)
    return _PROGRAM_CACHE[key]


def make_in_map(b, kv, qv, flags, queries, keys, values, mask_1, mask_2,
                Wq, Wk, Wv, Wo, d1_w, d1_b, d2_w, d2_b, ln_g, ln_b):
    has_d1b, has_d2b, has_g, has_b = flags
    W = min(qv + 1, L)
    kt_n = (kv + 127) // 128
    f32, bf16 = np.float32, ml_dtypes.bfloat16
    vl1 = int(np.count_nonzero(mask_1[b]))
    vl2 = int(np.count_nonzero(mask_2[b]))
    row01 = (np.arange(L) < vl2).astype(f32)
    qmT = (np.asarray(queries[b], f32) * row01[:, None]).T
    vb = np.asarray(values[b], f32)
    vaux_m = vb[vl1:, :].sum(0, dtype=np.float64).astype(f32)
    vTf = np.concatenate([vb.T[:, :kv], vaux_m[:, None]], axis=1)
    cnp = np.arange(128 * kt_n).reshape(kt_n, 128).T
    cn = np.where(cnp < vl1, 0.0, NEG).astype(f32)
    m = {
        "qT": np.ascontiguousarray(qmT[:, :W]).astype(bf16),
        "kT": np.ascontiguousarray(np.asarray(keys[b], f32).T[:, :kv]
                                   ).astype(bf16),
        "vT": np.ascontiguousarray(vTf).astype(bf16),
        "wq": (np.asarray(Wq, f32) * 0.125).astype(bf16),
        "wk": np.asarray(Wk, f32).astype(bf16),
        "wv": np.asarray(Wv, f32).astype(bf16),
        "wo": np.asarray(Wo, f32).astype(bf16),
        "d1w": np.asarray(d1_w, f32).astype(bf16),
        "d2w": np.asarray(d2_w, f32).astype(bf16),
        "colneg": np.ascontiguousarray(cn),
        "colnegT": np.where(np.arange(128 * kt_n) < vl1, 0.0,
                            NEG)[None, :].astype(bf16),
        "wvec": (1.0 - row01)[None, :W].astype(bf16),
        "sigu": np.full((1, H), float(L - vl1), f32).astype(bf16),
    }
    if has_d1b:
        m["d1b"] = np.ascontiguousarray(
            np.asarray(d1_b, f32).reshape(DT, 128).T)
    if has_d2b:
        m["d2b"] = np.asarray(d2_b, f32)[None, :]
    if has_g:
        m["lng"] = np.asarray(ln_g, f32)[None, :]
    if has_b:
        m["lnb"] = np.asarray(ln_b, f32)[None, :]
    return m


def kernel(queries, keys, values, mask_1, mask_2,
           Wq, Wk, Wv, Wo, d1_w, d1_b, d2_w, d2_b, ln_g, ln_b):
    from concourse.bass_utils import run_bass_kernel_spmd

    queries = np.asarray(queries)
    B = queries.shape[0]
    vl1 = np.count_nonzero(np.asarray(mask_1), axis=1)
    vl2 = np.count_nonzero(np.asarray(mask_2), axis=1)
    kv = int(((max(vl1.max(), 256) + 31) // 32) * 32)
    qv = int(max(vl2.max(), 256))
    flags = (bool(np.any(np.asarray(d1_b))), bool(np.any(np.asarray(d2_b))),
             not np.all(np.asarray(ln_g) == 1.0),
             bool(np.any(np.asarray(ln_b))))
    nc = _get_program(kv, qv, B, *flags)
    in_maps = [
        make_in_map(b, kv, qv, flags, queries, keys, values, mask_1, mask_2,
                    Wq, Wk, Wv, Wo, d1_w, d1_b, d2_w, d2_b, ln_g, ln_b)
        for b in range(B)
    ]
    res = run_bass_kernel_spmd(nc, in_maps, list(range(B)))
    return np.stack([res.results[b]["out"] for b in range(B)], axis=0)
